# revision 43
# baseline (speedup 1.0000x reference)
"""MultiHeadCrossAttention kernel for 8 Trainium2 NeuronCores.

Reference computation (b=2, nq=nk=2048, d_model=512, h=8, hd=64):
    Q = split_heads(q @ Wq.T + bq); K, V likewise
    S = Q K^T * hd^-0.5 ; A = softmax(S, -1) * mask_head * diag(pearson)
    out = merge_heads(A @ V)

Sharding: 16 (batch, head) pairs -> 2 heads of one batch per core.

Only the *diagonal* of pearson_matrix is used, so it is extracted on the
host and folded into the mask.  The mask is transposed, diag-folded,
tiled to the exact per-iteration consumption order and cast to fp16 on
the host, so every device-side mask DMA is one contiguous 512 KiB read.

All matmul operands are fp16 (PE runs 16-bit at 4x the fp32 rate, and
fp16's 10 mantissa bits keep the error ~1e-3, far under the 2e-2 gate).
PSUM accumulation stays fp32.

Device layout per core ("k on partitions, q on free axis"):
    S^T[k,q]   = sum_d K^T[d,k] Q^T[d,q]       (TensorE, d=64 contraction)
    E^T        = exp(SCALING * S^T)            (ScalarE, PSUM->SBUF fp16)
    Z[q]      += ones^T @ E^T                  (TensorE, PSUM-accumulated,
                                                heads packed in col groups
                                                0/32 -> run concurrently)
    A^T        = E^T * maskT_folded            (VectorE, fp16 2x mode)
    agg^T[e,q]+= sum_k V[k,e] A^T[k,q]         (TensorE, PSUM-accumulated,
                                                heads col-packed 0/64)

The scalar-engine exp stream (~1.0us per [128,1024] tile, 64 tiles) is
the critical path; everything else is scheduled around keeping it dense:

 * PSUM S tiles double-buffer per head so the S matmuls for k-tile kt+1
   are emitted right after the exp that frees the banks.
 * Z/AV matmul packs are emitted several iterations late (ZLAG/AVLAG) so
   the PE never blocks the S->exp chain on mask/projection stragglers.
 * The K(cols 1024:)/Q(cols 1024:)/V projections are interleaved into
   the early/mid iterations, borrowing the agg/z PSUM tags before the
   (lagged) first Z/AV accumulations claim them.
 * A burst of tiny warm-up matmuls keeps the PE HAM un-throttled while
   the k/q DMAs are in flight.
 * GpSimd is never given a data op, avoiding its ~6us one-time ucode
   IRAM load at kernel start.

The device returns unnormalized agg^T (128 rows = 2 heads x 64 dims) and
Z; the host divides, transposes and concatenates the 8 per-core slices.
"""

import ctypes
import os
import sys
import types

import numpy as np

import concourse.bacc as bacc
import concourse.bass as bass
import concourse.tile as tile
from concourse import mybir
from concourse.vector_clock import ScopedClock

F32 = mybir.dt.float32
F16 = mybir.dt.float16

B = 2
H = 8
N = 2048  # nq == nk
D = 512
HD = 64
HPC = 2  # heads per core
E = HPC * HD  # 128 output dims per core
SCALING = HD ** (-0.5)
NCORES = 8
P = 128
QC = 1024  # q super-chunk (2 per core)
NQC = N // QC
NKT = N // P  # 16 k tiles
NIT = NQC * NKT  # 32 global iterations
HF = 512  # matmul free-dim chunk (one PSUM bank)
NCC = D // P  # 4 contraction chunks for the projections

ZLAG = 5  # Z packs for iteration i are emitted at iteration i+ZLAG
AVLAG = 9  # AV pack lag (V projection borrows the agg banks early on)
MLEAD = 2  # mask DMA for iteration i is emitted at iteration i-MLEAD
NWARM = 34  # PE warm-up matmuls
HQ = 256  # Z/AV pack matmul free-dim (small => less PE head-of-line blocking)


# ---------------------------------------------------------------------------
# Page faults are extremely slow in this sandbox (~ms each); MAP_POPULATE
# prefaults an allocation in one syscall, ~100x faster for big arrays.
# ---------------------------------------------------------------------------
_libc = ctypes.CDLL(None, use_errno=True)
_libc.mmap.restype = ctypes.c_void_p
_libc.mmap.argtypes = [
    ctypes.c_void_p,
    ctypes.c_size_t,
    ctypes.c_int,
    ctypes.c_int,
    ctypes.c_int,
    ctypes.c_long,
]


def _alloc(shape, dtype=np.float32):
    nbytes = int(np.prod(shape)) * np.dtype(dtype).itemsize
    nbytes = (nbytes + 4095) & ~4095
    p = _libc.mmap(None, nbytes, 0x3, 0x02 | 0x20 | 0x8000, -1, 0)  # RW, PRIV|ANON|POPULATE
    if p in (None, ctypes.c_void_p(-1).value):
        return np.empty(shape, dtype)
    buf = (ctypes.c_byte * nbytes).from_address(p)
    return np.frombuffer(buf, dtype=dtype, count=int(np.prod(shape))).reshape(shape)


def _tcopy16(src):
    """Contiguous fp16 transposed copy of a 2-D array into prefaulted memory."""
    dst = _alloc((src.shape[1], src.shape[0]), np.float16)
    np.copyto(dst, src.T)
    return dst


# ---------------------------------------------------------------------------
# Environment shim: walrus in this container rejects >1 sync wait on
# CTRL-class instructions (NoOp/Drain), but TileContext's kernel-tail drain
# carries one wait per live semaphore.  Re-emit them as individual wait_ge
# instructions (one wait each) before a bare drain.
# ---------------------------------------------------------------------------
def _drain_and_barrier(self, tick_clock, wait_clock):
    probe = mybir.InstNoOp(
        name="wait_probe", ins=[], outs=[], engine=mybir.EngineType.SP
    )
    wait_clock.add_sem_waits(probe, ScopedClock({None: tick_clock.global_clock}))
    waits = list(probe.sync_info.on_wait) if probe.sync_info else []
    allocated = self.sems.allocated()
    by_name = {}
    for k, h in allocated.items():
        by_name[getattr(h, "name", str(k))] = h
    for w in waits:
        h = by_name.get(w.ant_name)
        assert h is not None, (w.ant_name, sorted(by_name))
        self.nc.sync.wait_ge(h, w.wait_value)
    self.nc.sync.drain()
    self.nc.all_engine_barrier()
    popped = self.nc._tile_sem_poison_stack.pop()
    assert popped is self._sem_poison
    self.nc.clear_and_free_semaphores(list(allocated.values()))
    self.nc.all_engine_barrier()


def _install_shims():
    tile.TileContext._drain_and_barrier = _drain_and_barrier
    if "antenv.axon_hooks" not in sys.modules:
        try:
            from trn_agent_boot.trn_boot import _ntff_profile_via_ctypes

            mod = types.ModuleType("antenv.axon_hooks")
            hook = _ntff_profile_via_ctypes("/opt/axon/libaxon_pjrt.so")
            mod.get_axon_ntff_profile_hook = lambda: hook
            mod.set_axon_ntff_profile_hook = lambda h: None
            sys.modules["antenv.axon_hooks"] = mod
        except Exception:
            pass


# ---------------------------------------------------------------------------
# Device kernel (one Bass program, SPMD over 8 cores; shards via in_maps)
# ---------------------------------------------------------------------------
def build_nc() -> bass.Bass:
    nc = bacc.Bacc("TRN2", target_bir_lowering=False, debug=False)

    # [half, d, n-half]: each column-half contiguous for full-rate DMA
    qTh = nc.dram_tensor("qTh", [2, D, QC], F16, kind="ExternalInput")
    kTh = nc.dram_tensor("kTh", [2, D, QC], F16, kind="ExternalInput")
    vTh = nc.dram_tensor("vTh", [2, D, QC], F16, kind="ExternalInput")
    # weights pre-tiled on the host to the SBUF layout [p, c, e]
    wq = nc.dram_tensor("wq", [P, NCC, E], F16, kind="ExternalInput")
    wk = nc.dram_tensor("wk", [P, NCC, E], F16, kind="ExternalInput")
    wv = nc.dram_tensor("wv", [P, NCC, E], F16, kind="ExternalInput")
    bqk = nc.dram_tensor("bqk", [E, 2], F32, kind="ExternalInput")  # bq|bk
    bvb = nc.dram_tensor("bvb", [P, E], F32, kind="ExternalInput")  # bv bcast
    onesd = nc.dram_tensor("onesd", [P, P], F16, kind="ExternalInput")
    # maskt[qc, kt, k, lh, q] = mask[b, h0+lh, qc*QC+q, kt*P+k]
    #                          * diag(pearson)[b, h0+lh, kt*P+k]
    maskt = nc.dram_tensor("maskt", [NQC, NKT, P, HPC, QC], F16, kind="ExternalInput")
    outT = nc.dram_tensor("outT", [E, N], F32, kind="ExternalOutput")
    # softmax denominators, normalization happens on the host
    zout = nc.dram_tensor("zout", [HPC, N], F32, kind="ExternalOutput")

    with tile.TileContext(nc) as tc:
        with (
            tc.tile_pool(name="consts", bufs=1) as consts,
            tc.tile_pool(name="persist", bufs=1) as persist,
            tc.tile_pool(name="ps", bufs=1, space="PSUM") as ps,
            tc.tile_pool(name="qkv", bufs=1) as qkv,
            tc.tile_pool(name="et", bufs=ZLAG + 3) as etp,
            tc.tile_pool(name="at", bufs=AVLAG + 2) as atp,
            tc.tile_pool(name="mask", bufs=MLEAD + 4) as maskp,
            tc.tile_pool(name="small", bufs=2) as smallp,
            tc.tile_pool(name="outp", bufs=2) as outp,
        ):
            def s_tile(sl, name):
                return ps.tile([P, QC], F32, tag=f"s{sl}", name=name)

            def agg_tag_tile(name, shape=(P, QC)):
                return ps.tile(list(shape), F32, tag="agg", name=name)

            def z_tag_tile(name, shape=(33, QC)):
                return ps.tile(list(shape), F32, tag="z", name=name)

            # ---- constants & input DMAs (ordering = HWDGE FIFO order) -----
            ones = consts.tile([P, P], F16, tag="ones")
            nc.sync.dma_start(out=ones, in_=onesd[:, :])
            wk_sb = consts.tile([P, NCC, E], F16, tag="wk")
            nc.sync.dma_start(out=wk_sb, in_=wk[:, :, :])
            bqk_sb = consts.tile([E, 2], F32, tag="bqk")
            nc.sync.dma_start(out=bqk_sb, in_=bqk[:, :])
            wq_sb = consts.tile([P, NCC, E], F16, tag="wq")
            nc.sync.dma_start(out=wq_sb, in_=wq[:, :, :])

            # k/q staged as column-half chunks so the projections feeding the
            # exp stream's first iterations wait on 1 MiB, not 2 MiB, each.
            # One 1-MiB DMA per half (not 4x 256 KiB) for full DMA rate.
            ka_t = qkv.tile([P, NCC, QC], F16, tag="ka")
            qa_t = qkv.tile([P, NCC, QC], F16, tag="qa")
            kb_t = qkv.tile([P, NCC, QC], F16, tag="kb")
            qb_t = qkv.tile([P, NCC, QC], F16, tag="qb")
            v_t = qkv.tile([P, NCC, 2, QC], F16, tag="v")
            nc.sync.dma_start(out=ka_t, in_=kTh[0].rearrange("(c p) n -> p c n", p=P))
            nc.sync.dma_start(out=qa_t, in_=qTh[0].rearrange("(c p) n -> p c n", p=P))
            nc.sync.dma_start(out=kb_t, in_=kTh[1].rearrange("(c p) n -> p c n", p=P))
            nc.sync.dma_start(out=qb_t, in_=qTh[1].rearrange("(c p) n -> p c n", p=P))
            wv_sb = consts.tile([P, NCC, E], F16, tag="wv")
            bvb_sb = consts.tile([P, E], F32, tag="bvb")
            for vh in range(2):
                nc.sync.dma_start(
                    out=v_t[:, :, vh, :],
                    in_=vTh[vh].rearrange("(c p) n -> p c n", p=P),
                )
            nc.sync.dma_start(out=wv_sb, in_=wv[:, :, :])
            nc.sync.dma_start(out=bvb_sb, in_=bvb[:, :])
            kas = [ka_t[:, c, :] for c in range(NCC)]
            qas = [qa_t[:, c, :] for c in range(NCC)]
            kbs = [kb_t[:, c, :] for c in range(NCC)]
            qbs = [qb_t[:, c, :] for c in range(NCC)]
            vhs = [v_t[:, c, vh, :] for c in range(NCC) for vh in range(2)]

            # Preload the exp spline table during the DMAs.
            scratch = consts.tile([P, 8], F32, tag="scratch")
            nc.scalar.activation(scratch, ones[:, 0:8], mybir.ActivationFunctionType.Exp)

            # PE warm-up: a dense burst of tiny matmuls while k/q stream in,
            # so the HAM clock gate opens before the projections run.
            warm_ps = s_tile(0, "warm_ps")
            for i in range(NWARM):
                nc.tensor.matmul(
                    warm_ps[0:1, 0:P],
                    ones[:, 0:1],
                    ones[:, :],
                    start=True,
                    stop=True,
                    skip_group_check=True,
                )

            # ---- K/Q projections, first halves (cols 0:1024) --------------
            # [e, n] = sum_c w[c, e] * xT[c, n]; bias added during the
            # PSUM->SBUF eviction on VectorE (per-partition scalar operand).
            QT_sb = persist.tile([E, N], F16, tag="QT")  # [e, n] 2 heads x 64
            KT_sb = persist.tile([E, N], F16, tag="KT")
            V_sb = persist.tile([P, NKT, E], F16, tag="V")  # [k%128, kt, e]

            def emit_proj(dst, w_sb, bias_ap, srcs, jcol, pst):
                for half in range(QC // HF):
                    cols = slice(half * HF, (half + 1) * HF)
                    for c in range(NCC):
                        nc.tensor.matmul(
                            pst[:, half * HF : (half + 1) * HF],
                            w_sb[:, c, :],
                            srcs[c][:, cols],
                            start=(c == 0),
                            stop=(c == NCC - 1),
                        )
                nc.vector.tensor_scalar_add(
                    dst[:, jcol * QC : (jcol + 1) * QC], pst, bias_ap
                )

            emit_proj(KT_sb, wk_sb, bqk_sb[:, 1:2], kas, 0, s_tile(0, "ps_kA"))
            emit_proj(QT_sb, wq_sb, bqk_sb[:, 0:1], qas, 0, s_tile(1, "ps_qA"))

            def emit_v_chunk(sub, pst):
                """V natural layout: [n, e] = sum_c vT[c, n] * w[c, e].
                One sub-chunk = 4 k-tiles into half of a [128,1024] psum."""
                for t4 in range(4):
                    t = sub * 4 + t4
                    col = (sub % 2) * 4 + t4
                    for c in range(NCC):
                        nc.tensor.matmul(
                            pst[:, col * E : (col + 1) * E],
                            vhs[2 * c + t // 8][:, (t % 8) * P : (t % 8 + 1) * P],
                            wv_sb[:, c, :],
                            start=(c == 0),
                            stop=(c == NCC - 1),
                        )
                for t4 in range(4):
                    t = sub * 4 + t4
                    col = (sub % 2) * 4 + t4
                    nc.vector.tensor_add(
                        V_sb[:, t, :], pst[:, col * E : (col + 1) * E], bvb_sb
                    )

            # ---- attention: 32 software-pipelined iterations ---------------
            def emit_s(it, lh, s_ps):
                qc, kt = divmod(it, NKT)
                kcols = slice(kt * P, (kt + 1) * P)
                hsl = slice(lh * HD, (lh + 1) * HD)
                for half in range(QC // HF):
                    rcols = slice(qc * QC + half * HF, qc * QC + (half + 1) * HF)
                    nc.tensor.matmul(
                        s_ps[:, half * HF : (half + 1) * HF],
                        KT_sb[hsl, kcols],
                        QT_sb[hsl, rcols],
                        start=True,
                        stop=True,
                        tile_position=(lh * HD, 0),
                    )

            ets = {}
            ats = {}
            aggs = {}
            zpss = {}

            def emit_mask_dma(it):
                qc, kt = divmod(it, NKT)
                mt = maskp.tile([P, HPC, QC], F16, tag="mt", name=f"mt_{it}")
                nc.sync.dma_start(out=mt, in_=maskt[qc, kt])
                return mt

            masks = {it: emit_mask_dma(it) for it in range(MLEAD)}

            def emit_z_pack(it):
                qc, kt = divmod(it, NKT)
                if kt == 0:
                    zpss[qc] = z_tag_tile(f"z{qc}")
                zps, et = zpss[qc], ets.pop(it)
                for half in range(QC // HQ):
                    hcols = slice(half * HQ, (half + 1) * HQ)
                    # start=True clears the has_written bits of the WHOLE
                    # psum bank (for the written partitions), so only the
                    # first sub-bank group may carry it; the second group's
                    # kt==0 matmul overwrites thanks to the cleared bits.
                    first = kt == 0 and (half * HQ) % HF == 0
                    for lh in range(HPC):
                        nc.tensor.matmul(
                            zps[lh * 32 : lh * 32 + 1, hcols],
                            ones[:, 0:1],
                            et[:, lh, hcols],
                            start=first,
                            stop=(kt == NKT - 1),
                            tile_position=(0, lh * 32),
                            skip_group_check=True,
                        )

            def emit_av_pack(it):
                qc, kt = divmod(it, NKT)
                if kt == 0:
                    aggs[qc] = agg_tag_tile(f"agg{qc}")
                agg, at = aggs[qc], ats.pop(it)
                for half in range(QC // HQ):
                    hcols = slice(half * HQ, (half + 1) * HQ)
                    first = kt == 0 and (half * HQ) % HF == 0
                    for lh in range(HPC):
                        esl = slice(lh * HD, (lh + 1) * HD)
                        nc.tensor.matmul(
                            agg[esl, hcols],
                            V_sb[:, kt, esl],
                            at[:, lh, hcols],
                            start=first,
                            stop=(kt == NKT - 1),
                            tile_position=(0, lh * HD),
                            skip_group_check=True,
                        )

            def emit_epilogue(qc):
                qcols = slice(qc * QC, (qc + 1) * QC)
                zps, agg = zpss.pop(qc), aggs.pop(qc)
                zsb = smallp.tile([33, QC], F32, tag="zsb", name=f"zsb{qc}")
                nc.vector.tensor_copy(zsb, zps)
                for lh in range(HPC):
                    nc.sync.dma_start(
                        out=zout[lh, qcols], in_=zsb[lh * 32 : lh * 32 + 1, :]
                    )
                # split the agg eviction so the first half's DMA overlaps the
                # second half's PSUM->SBUF copy
                osb = outp.tile([P, QC], F32, tag="osb", name=f"osb_{qc}")
                for half in range(2):
                    hsl = slice(half * HF, (half + 1) * HF)
                    nc.vector.tensor_copy(osb[:, hsl], agg[:, hsl])
                    nc.sync.dma_start(
                        out=outT[:, qc * QC + half * HF : qc * QC + (half + 1) * HF],
                        in_=osb[:, hsl],
                    )

            s_cur = [s_tile(lh, f"s_0_{lh}") for lh in range(HPC)]
            for lh in range(HPC):
                emit_s(0, lh, s_cur[lh])

            z_next = 0
            av_next = 0
            epi_done = 0
            v_ps = [None, None]  # borrowed psum tiles for the V sub-chunks

            def try_epi():
                nonlocal epi_done
                while (
                    epi_done < NQC
                    and z_next > epi_done * NKT + NKT - 1
                    and av_next > epi_done * NKT + NKT - 1
                ):
                    emit_epilogue(epi_done)
                    epi_done += 1

            def drain_packs(it):
                """Emit pending Z/AV packs.  Z runs ZLAG behind (gated only
                by the K-B/Q-B psum borrow of the z banks); AV runs AVLAG
                behind (the V projection borrows the agg banks) and catches
                up from it>=14 so both cross the qc boundary together.  A
                pack may not enter a new qc until that qc's predecessor
                epilogue is out (its first matmul re-claims the banks the
                epilogue eviction reads)."""
                nonlocal z_next, av_next
                if it >= 30:
                    zlim, zq = it, 2
                elif it >= 26:
                    zlim, zq = it - 1, 2
                else:
                    zlim, zq = it - ZLAG, 1
                for _ in range(zq):
                    j = z_next
                    if j >= NIT or j > zlim:
                        break
                    if j % NKT == 0 and j > 0 and epi_done < j // NKT:
                        break
                    emit_z_pack(j)
                    z_next += 1
                    try_epi()
                if it >= 30:
                    alim, aq = it, 2
                elif it >= 24:
                    alim, aq = it - 1, 2
                elif it >= 14:
                    alim, aq = it - 2, (2 if it % 2 == 0 else 1)
                else:
                    alim, aq = it - AVLAG, 1
                for _ in range(aq):
                    j = av_next
                    if j >= NIT or j > alim:
                        break
                    if j % NKT == 0 and j > 0 and epi_done < j // NKT:
                        break
                    emit_av_pack(j)
                    av_next += 1
                    try_epi()

            for it in range(NIT):
                last = it == NIT - 1
                if it + MLEAD < NIT:
                    masks[it + MLEAD] = emit_mask_dma(it + MLEAD)
                mt = masks.pop(it)
                et = etp.tile([P, HPC, QC], F16, tag="et", name=f"et_{it}")
                at = atp.tile([P, HPC, QC], F16, tag="at", name=f"at_{it}")
                ets[it], ats[it] = et, at
                s_nxt = (
                    [s_tile(lh, f"s_{it + 1}_{lh}") for lh in range(HPC)]
                    if not last
                    else None
                )
                for lh in range(HPC):
                    nc.scalar.activation(
                        et[:, lh, :],
                        s_cur[lh],
                        mybir.ActivationFunctionType.Exp,
                        scale=SCALING,
                    )
                    nc.vector.tensor_mul(at[:, lh, :], et[:, lh, :], mt[:, lh, :])
                    # S^T for the next iteration reuses this head's PSUM
                    # banks; emit right after the exp that frees them.
                    if not last:
                        emit_s(it + 1, lh, s_nxt[lh])
                s_cur = s_nxt

                # deferred projections, wedged into the PE's slack BEFORE the
                # Z/AV packs (the packs may wait on late masks; the wedges
                # must not queue behind them).  Each wedge's inputs are in
                # SBUF just before the PE's FIFO reaches it, so it never
                # blocks the S matmuls emitted after it.
                if it == 1:
                    emit_proj(KT_sb, wk_sb, bqk_sb[:, 1:2], kbs, 1,
                              z_tag_tile("ps_kB", shape=(P, QC)))
                elif it == 3:
                    emit_proj(QT_sb, wq_sb, bqk_sb[:, 0:1], qbs, 1,
                              z_tag_tile("ps_qB", shape=(P, QC)))
                elif it == 4:
                    v_ps[0] = agg_tag_tile("ps_vA")
                    emit_v_chunk(0, v_ps[0])
                elif it == 5:
                    emit_v_chunk(1, v_ps[0])
                elif it == 6:
                    v_ps[1] = agg_tag_tile("ps_vB")
                    emit_v_chunk(2, v_ps[1])
                elif it == 7:
                    emit_v_chunk(3, v_ps[1])

                drain_packs(it)

            it = NIT
            while z_next < NIT or av_next < NIT:
                drain_packs(it)
                it += 1

    nc.compile()
    return nc


# ---------------------------------------------------------------------------
# Host side
# ---------------------------------------------------------------------------
def _prep_in_maps(q, k, v, mask_head, pearson_matrix, Wq, bq, Wk, bk, Wv, bv):
    f = np.float32
    q = np.asarray(q, f)
    k = np.asarray(k, f)
    v = np.asarray(v, f)
    mask_head = np.asarray(mask_head, f)
    Wq = np.asarray(Wq, f)
    Wk = np.asarray(Wk, f)
    Wv = np.asarray(Wv, f)
    bq = np.asarray(bq, f).reshape(D)
    bk = np.asarray(bk, f).reshape(D)
    bv = np.asarray(bv, f).reshape(D)

    # Only the diagonal of pearson is used by the computation.
    pm = np.asarray(pearson_matrix)
    diag = np.ascontiguousarray(np.diagonal(pm, axis1=-2, axis2=-1)).astype(f)

    def _thalves(x):
        """x [n, d] -> [2, d, n/2] fp16, each column-half contiguous."""
        dst = _alloc((2, D, QC), np.float16)
        xT = x.T
        np.copyto(dst[0], xT[:, 0:QC])
        np.copyto(dst[1], xT[:, QC:N])
        return dst

    qT = [_thalves(q[b]) for b in range(B)]
    kTt = [_thalves(k[b]) for b in range(B)]
    vTt = [_thalves(v[b]) for b in range(B)]
    onesd = np.ones((P, P), np.float16)

    def wtile(W, esl):
        # [D, E] -> [P, NCC, E] with d = c*P + p
        wT = np.ascontiguousarray(W[esl, :].T.astype(np.float16))
        return np.ascontiguousarray(wT.reshape(NCC, P, E).transpose(1, 0, 2))

    # Per-(b,h) mask, transposed to [k, q], diag-folded, tiled to the exact
    # per-iteration consumption order: [qc, kt, k, lh, q].
    maskt_all = _alloc((B, H // HPC, NQC, NKT, P, HPC, QC), np.float16)
    for b in range(B):
        for h in range(H):
            md = mask_head[b, h].T * diag[b, h][:, None]  # [k, q] f32
            tiled = md.reshape(NKT, P, NQC, QC).transpose(2, 0, 1, 3)
            maskt_all[b, h // HPC, :, :, :, h % HPC, :] = tiled

    in_maps = []
    for c in range(NCORES):
        b = c // (NCORES // B)
        h0 = HPC * (c % (NCORES // B))
        esl = slice(h0 * HD, (h0 + HPC) * HD)
        bqk = np.ascontiguousarray(
            np.stack([bq[esl], bk[esl]], axis=1).astype(f)
        )
        bvb = np.ascontiguousarray(
            np.broadcast_to(bv[esl][None, :], (P, E)).astype(f)
        )
        in_maps.append(
            {
                "qTh": qT[b],
                "kTh": kTt[b],
                "vTh": vTt[b],
                "wq": wtile(Wq, esl),
                "wk": wtile(Wk, esl),
                "wv": wtile(Wv, esl),
                "bqk": bqk,
                "bvb": bvb,
                "onesd": onesd,
                "maskt": maskt_all[b, h0 // HPC],
            }
        )
    return in_maps


_NC_CACHE = None
LAST_RESULT = None  # BassKernelResults of the most recent run (for profiling)


def kernel(**inputs) -> np.ndarray:
    global _NC_CACHE, LAST_RESULT
    _install_shims()
    from concourse.bass_utils import run_bass_kernel_spmd

    if _NC_CACHE is None:
        _NC_CACHE = build_nc()
    nc = _NC_CACHE

    in_maps = _prep_in_maps(**inputs)

    trace = bool(int(os.environ.get("KERNEL_TRACE", "0")))
    kwargs = {}
    if trace:
        kwargs["trace"] = True
        tmpdir = os.environ.get("KERNEL_TRACE_DIR")
        if tmpdir:
            kwargs["tmpdir"] = tmpdir
    res = run_bass_kernel_spmd(nc, in_maps, list(range(NCORES)), **kwargs)
    LAST_RESULT = res

    out = _alloc((B, N, D), np.float32)
    for c in range(NCORES):
        b = c // (NCORES // B)
        h0 = HPC * (c % (NCORES // B))
        aggT = res.results[c]["outT"]  # (E, N) unnormalized
        z = res.results[c]["zout"]  # (HPC, N)
        out[b, :, h0 * HD : (h0 + HPC) * HD] = (
            aggT / np.repeat(z, HD, axis=0)
        ).T
    return out


# revision 44
# speedup vs baseline: 1.0316x; 1.0316x over previous
"""MultiHeadCrossAttention kernel for 8 Trainium2 NeuronCores.

Reference computation (b=2, nq=nk=2048, d_model=512, h=8, hd=64):
    Q = split_heads(q @ Wq.T + bq); K, V likewise
    S = Q K^T * hd^-0.5 ; A = softmax(S, -1) * mask_head * diag(pearson)
    out = merge_heads(A @ V)

Sharding: 16 (batch, head) pairs -> 2 heads of one batch per core.

Only the *diagonal* of pearson_matrix is used, so it is extracted on the
host and folded into the mask.  The mask is transposed, diag-folded,
tiled to the exact per-iteration consumption order and cast to fp16 on
the host, so every device-side mask DMA is one contiguous 512 KiB read.

All matmul operands are fp16 (PE runs 16-bit at 4x the fp32 rate, and
fp16's 10 mantissa bits keep the error ~1e-3, far under the 2e-2 gate).
PSUM accumulation stays fp32.

Device layout per core ("k on partitions, q on free axis"):
    S^T[k,q]   = sum_d K^T[d,k] Q^T[d,q]       (TensorE, d=64 contraction)
    E^T        = exp(SCALING * S^T)            (ScalarE, PSUM->SBUF fp16)
    Z[q]      += ones^T @ E^T                  (TensorE, PSUM-accumulated,
                                                heads packed in col groups
                                                0/32 -> run concurrently)
    A^T        = E^T * maskT_folded            (VectorE, fp16 2x mode)
    agg^T[e,q]+= sum_k V[k,e] A^T[k,q]         (TensorE, PSUM-accumulated,
                                                heads col-packed 0/64)

The scalar-engine exp stream (~1.0us per [128,1024] tile, 64 tiles) is
the critical path; everything else is scheduled around keeping it dense:

 * PSUM S tiles double-buffer per head so the S matmuls for k-tile kt+1
   are emitted right after the exp that frees the banks.
 * Z/AV matmul packs are emitted several iterations late (ZLAG/AVLAG) so
   the PE never blocks the S->exp chain on mask/projection stragglers.
 * The K(cols 1024:)/Q(cols 1024:)/V projections are interleaved into
   the early/mid iterations, borrowing the agg/z PSUM tags before the
   (lagged) first Z/AV accumulations claim them.
 * A burst of tiny warm-up matmuls keeps the PE HAM un-throttled while
   the k/q DMAs are in flight.
 * GpSimd is never given a data op, avoiding its ~6us one-time ucode
   IRAM load at kernel start.

The device returns unnormalized agg^T (128 rows = 2 heads x 64 dims) and
Z; the host divides, transposes and concatenates the 8 per-core slices.
"""

import ctypes
import os
import sys
import types

import numpy as np

import concourse.bacc as bacc
import concourse.bass as bass
import concourse.tile as tile
from concourse import mybir
from concourse.vector_clock import ScopedClock

F32 = mybir.dt.float32
F16 = mybir.dt.float16

B = 2
H = 8
N = 2048  # nq == nk
D = 512
HD = 64
HPC = 2  # heads per core
E = HPC * HD  # 128 output dims per core
SCALING = HD ** (-0.5)
NCORES = 8
P = 128
QC = 1024  # q super-chunk (2 per core)
NQC = N // QC
NKT = N // P  # 16 k tiles
NIT = NQC * NKT  # 32 global iterations
HF = 512  # matmul free-dim chunk (one PSUM bank)
NCC = D // P  # 4 contraction chunks for the projections

ZLAG = 5  # Z packs for iteration i are emitted at iteration i+ZLAG
AVLAG = 9  # AV pack lag (V projection borrows the agg banks early on)
MLEAD = 2  # mask DMA for iteration i is emitted at iteration i-MLEAD
NWARM = 56  # PE warm-up matmuls
HQ = 256  # Z/AV pack matmul free-dim (small => less PE head-of-line blocking)


# ---------------------------------------------------------------------------
# Page faults are extremely slow in this sandbox (~ms each); MAP_POPULATE
# prefaults an allocation in one syscall, ~100x faster for big arrays.
# ---------------------------------------------------------------------------
_libc = ctypes.CDLL(None, use_errno=True)
_libc.mmap.restype = ctypes.c_void_p
_libc.mmap.argtypes = [
    ctypes.c_void_p,
    ctypes.c_size_t,
    ctypes.c_int,
    ctypes.c_int,
    ctypes.c_int,
    ctypes.c_long,
]


def _alloc(shape, dtype=np.float32):
    nbytes = int(np.prod(shape)) * np.dtype(dtype).itemsize
    nbytes = (nbytes + 4095) & ~4095
    p = _libc.mmap(None, nbytes, 0x3, 0x02 | 0x20 | 0x8000, -1, 0)  # RW, PRIV|ANON|POPULATE
    if p in (None, ctypes.c_void_p(-1).value):
        return np.empty(shape, dtype)
    buf = (ctypes.c_byte * nbytes).from_address(p)
    return np.frombuffer(buf, dtype=dtype, count=int(np.prod(shape))).reshape(shape)


def _tcopy16(src):
    """Contiguous fp16 transposed copy of a 2-D array into prefaulted memory."""
    dst = _alloc((src.shape[1], src.shape[0]), np.float16)
    np.copyto(dst, src.T)
    return dst


# ---------------------------------------------------------------------------
# Environment shim: walrus in this container rejects >1 sync wait on
# CTRL-class instructions (NoOp/Drain), but TileContext's kernel-tail drain
# carries one wait per live semaphore.  Re-emit them as individual wait_ge
# instructions (one wait each) before a bare drain.
# ---------------------------------------------------------------------------
def _drain_and_barrier(self, tick_clock, wait_clock):
    probe = mybir.InstNoOp(
        name="wait_probe", ins=[], outs=[], engine=mybir.EngineType.SP
    )
    wait_clock.add_sem_waits(probe, ScopedClock({None: tick_clock.global_clock}))
    waits = list(probe.sync_info.on_wait) if probe.sync_info else []
    allocated = self.sems.allocated()
    by_name = {}
    for k, h in allocated.items():
        by_name[getattr(h, "name", str(k))] = h
    for w in waits:
        h = by_name.get(w.ant_name)
        assert h is not None, (w.ant_name, sorted(by_name))
        self.nc.sync.wait_ge(h, w.wait_value)
    self.nc.sync.drain()
    self.nc.all_engine_barrier()
    popped = self.nc._tile_sem_poison_stack.pop()
    assert popped is self._sem_poison
    self.nc.clear_and_free_semaphores(list(allocated.values()))
    self.nc.all_engine_barrier()


def _install_shims():
    tile.TileContext._drain_and_barrier = _drain_and_barrier
    if "antenv.axon_hooks" not in sys.modules:
        try:
            from trn_agent_boot.trn_boot import _ntff_profile_via_ctypes

            mod = types.ModuleType("antenv.axon_hooks")
            hook = _ntff_profile_via_ctypes("/opt/axon/libaxon_pjrt.so")
            mod.get_axon_ntff_profile_hook = lambda: hook
            mod.set_axon_ntff_profile_hook = lambda h: None
            sys.modules["antenv.axon_hooks"] = mod
        except Exception:
            pass


# ---------------------------------------------------------------------------
# Device kernel (one Bass program, SPMD over 8 cores; shards via in_maps)
# ---------------------------------------------------------------------------
def build_nc() -> bass.Bass:
    nc = bacc.Bacc("TRN2", target_bir_lowering=False, debug=False)

    # [half, d, n-half]: each column-half contiguous for full-rate DMA
    qTh = nc.dram_tensor("qTh", [2, D, QC], F16, kind="ExternalInput")
    kTh = nc.dram_tensor("kTh", [2, D, QC], F16, kind="ExternalInput")
    vTh = nc.dram_tensor("vTh", [2, D, QC], F16, kind="ExternalInput")
    # weights pre-tiled on the host to the SBUF layout [p, c, e]
    wq = nc.dram_tensor("wq", [P, NCC, E], F16, kind="ExternalInput")
    wk = nc.dram_tensor("wk", [P, NCC, E], F16, kind="ExternalInput")
    wv = nc.dram_tensor("wv", [P, NCC, E], F16, kind="ExternalInput")
    bqk = nc.dram_tensor("bqk", [E, 2], F32, kind="ExternalInput")  # bq|bk
    bvb = nc.dram_tensor("bvb", [P, E], F32, kind="ExternalInput")  # bv bcast
    onesd = nc.dram_tensor("onesd", [P, P], F16, kind="ExternalInput")
    # maskt[qc, kt, k, lh, q] = mask[b, h0+lh, qc*QC+q, kt*P+k]
    #                          * diag(pearson)[b, h0+lh, kt*P+k]
    maskt = nc.dram_tensor("maskt", [NQC, NKT, P, HPC, QC], F16, kind="ExternalInput")
    outT = nc.dram_tensor("outT", [E, N], F16, kind="ExternalOutput")
    # softmax denominators, normalization happens on the host
    zout = nc.dram_tensor("zout", [HPC, N], F16, kind="ExternalOutput")

    with tile.TileContext(nc) as tc:
        with (
            tc.tile_pool(name="consts", bufs=1) as consts,
            tc.tile_pool(name="persist", bufs=1) as persist,
            tc.tile_pool(name="ps", bufs=1, space="PSUM") as ps,
            tc.tile_pool(name="qkv", bufs=1) as qkv,
            tc.tile_pool(name="et", bufs=ZLAG + 3) as etp,
            tc.tile_pool(name="at", bufs=AVLAG + 2) as atp,
            tc.tile_pool(name="mask", bufs=MLEAD + 4) as maskp,
            tc.tile_pool(name="small", bufs=2) as smallp,
            tc.tile_pool(name="outp", bufs=2) as outp,
        ):
            def s_tile(sl, name):
                return ps.tile([P, QC], F32, tag=f"s{sl}", name=name)

            def agg_tag_tile(name, shape=(P, QC)):
                return ps.tile(list(shape), F32, tag="agg", name=name)

            def z_tag_tile(name, shape=(33, QC)):
                return ps.tile(list(shape), F32, tag="z", name=name)

            # ---- constants & input DMAs (ordering = HWDGE FIFO order) -----
            ones = consts.tile([P, P], F16, tag="ones")
            nc.sync.dma_start(out=ones, in_=onesd[:, :])
            wk_sb = consts.tile([P, NCC, E], F16, tag="wk")
            nc.sync.dma_start(out=wk_sb, in_=wk[:, :, :])
            bqk_sb = consts.tile([E, 2], F32, tag="bqk")
            nc.sync.dma_start(out=bqk_sb, in_=bqk[:, :])
            wq_sb = consts.tile([P, NCC, E], F16, tag="wq")
            nc.sync.dma_start(out=wq_sb, in_=wq[:, :, :])

            # k/q staged as column-half chunks so the projections feeding the
            # exp stream's first iterations wait on 1 MiB, not 2 MiB, each.
            # One 1-MiB DMA per half (not 4x 256 KiB) for full DMA rate.
            ka_t = qkv.tile([P, NCC, QC], F16, tag="ka")
            qa_t = qkv.tile([P, NCC, QC], F16, tag="qa")
            kb_t = qkv.tile([P, NCC, QC], F16, tag="kb")
            qb_t = qkv.tile([P, NCC, QC], F16, tag="qb")
            v_t = qkv.tile([P, NCC, 2, QC], F16, tag="v")
            nc.sync.dma_start(out=ka_t, in_=kTh[0].rearrange("(c p) n -> p c n", p=P))
            nc.sync.dma_start(out=qa_t, in_=qTh[0].rearrange("(c p) n -> p c n", p=P))
            nc.sync.dma_start(out=kb_t, in_=kTh[1].rearrange("(c p) n -> p c n", p=P))
            nc.sync.dma_start(out=qb_t, in_=qTh[1].rearrange("(c p) n -> p c n", p=P))
            wv_sb = consts.tile([P, NCC, E], F16, tag="wv")
            bvb_sb = consts.tile([P, E], F32, tag="bvb")
            for vh in range(2):
                nc.sync.dma_start(
                    out=v_t[:, :, vh, :],
                    in_=vTh[vh].rearrange("(c p) n -> p c n", p=P),
                )
            nc.sync.dma_start(out=wv_sb, in_=wv[:, :, :])
            nc.sync.dma_start(out=bvb_sb, in_=bvb[:, :])
            kas = [ka_t[:, c, :] for c in range(NCC)]
            qas = [qa_t[:, c, :] for c in range(NCC)]
            kbs = [kb_t[:, c, :] for c in range(NCC)]
            qbs = [qb_t[:, c, :] for c in range(NCC)]
            vhs = [v_t[:, c, vh, :] for c in range(NCC) for vh in range(2)]

            # Preload the exp spline table during the DMAs.
            scratch = consts.tile([P, 8], F32, tag="scratch")
            nc.scalar.activation(scratch, ones[:, 0:8], mybir.ActivationFunctionType.Exp)

            # PE warm-up: a dense burst of tiny matmuls while k/q stream in,
            # so the HAM clock gate opens before the projections run.
            warm_ps = s_tile(0, "warm_ps")
            for i in range(NWARM):
                nc.tensor.matmul(
                    warm_ps[0:1, 0:P],
                    ones[:, 0:1],
                    ones[:, :],
                    start=True,
                    stop=True,
                    skip_group_check=True,
                )

            # ---- K/Q projections, first halves (cols 0:1024) --------------
            # [e, n] = sum_c w[c, e] * xT[c, n]; bias added during the
            # PSUM->SBUF eviction on VectorE (per-partition scalar operand).
            QT_sb = persist.tile([E, N], F16, tag="QT")  # [e, n] 2 heads x 64
            KT_sb = persist.tile([E, N], F16, tag="KT")
            V_sb = persist.tile([P, NKT, E], F16, tag="V")  # [k%128, kt, e]

            def emit_proj(dst, w_sb, bias_ap, srcs, jcol, pst):
                for half in range(QC // HF):
                    cols = slice(half * HF, (half + 1) * HF)
                    for c in range(NCC):
                        nc.tensor.matmul(
                            pst[:, half * HF : (half + 1) * HF],
                            w_sb[:, c, :],
                            srcs[c][:, cols],
                            start=(c == 0),
                            stop=(c == NCC - 1),
                        )
                for half in range(QC // HF):
                    cols = slice(jcol * QC + half * HF, jcol * QC + (half + 1) * HF)
                    nc.vector.tensor_scalar_add(
                        dst[:, cols], pst[:, half * HF : (half + 1) * HF], bias_ap
                    )

            emit_proj(KT_sb, wk_sb, bqk_sb[:, 1:2], kas, 0, s_tile(0, "ps_kA"))
            emit_proj(QT_sb, wq_sb, bqk_sb[:, 0:1], qas, 0, s_tile(1, "ps_qA"))

            def emit_v_chunk(sub, pst):
                """V natural layout: [n, e] = sum_c vT[c, n] * w[c, e].
                One sub-chunk = 4 k-tiles into half of a [128,1024] psum."""
                for t4 in range(4):
                    t = sub * 4 + t4
                    col = (sub % 2) * 4 + t4
                    for c in range(NCC):
                        nc.tensor.matmul(
                            pst[:, col * E : (col + 1) * E],
                            vhs[2 * c + t // 8][:, (t % 8) * P : (t % 8 + 1) * P],
                            wv_sb[:, c, :],
                            start=(c == 0),
                            stop=(c == NCC - 1),
                        )
                for t4 in range(4):
                    t = sub * 4 + t4
                    col = (sub % 2) * 4 + t4
                    nc.vector.tensor_add(
                        V_sb[:, t, :], pst[:, col * E : (col + 1) * E], bvb_sb
                    )

            # ---- attention: 32 software-pipelined iterations ---------------
            def emit_s(it, lh, s_ps):
                qc, kt = divmod(it, NKT)
                kcols = slice(kt * P, (kt + 1) * P)
                hsl = slice(lh * HD, (lh + 1) * HD)
                for half in range(QC // HF):
                    rcols = slice(qc * QC + half * HF, qc * QC + (half + 1) * HF)
                    nc.tensor.matmul(
                        s_ps[:, half * HF : (half + 1) * HF],
                        KT_sb[hsl, kcols],
                        QT_sb[hsl, rcols],
                        start=True,
                        stop=True,
                        tile_position=(lh * HD, 0),
                    )

            ets = {}
            ats = {}
            aggs = {}
            zpss = {}

            def emit_mask_dma(it):
                qc, kt = divmod(it, NKT)
                mt = maskp.tile([P, HPC, QC], F16, tag="mt", name=f"mt_{it}")
                nc.sync.dma_start(out=mt, in_=maskt[qc, kt])
                return mt

            masks = {it: emit_mask_dma(it) for it in range(MLEAD)}

            def emit_z_pack(it):
                qc, kt = divmod(it, NKT)
                if kt == 0:
                    zpss[qc] = z_tag_tile(f"z{qc}")
                zps, et = zpss[qc], ets.pop(it)
                for half in range(QC // HQ):
                    hcols = slice(half * HQ, (half + 1) * HQ)
                    # start=True clears the has_written bits of the WHOLE
                    # psum bank (for the written partitions), so only the
                    # first sub-bank group may carry it; the second group's
                    # kt==0 matmul overwrites thanks to the cleared bits.
                    first = kt == 0 and (half * HQ) % HF == 0
                    for lh in range(HPC):
                        nc.tensor.matmul(
                            zps[lh * 32 : lh * 32 + 1, hcols],
                            ones[:, 0:1],
                            et[:, lh, hcols],
                            start=first,
                            stop=(kt == NKT - 1),
                            tile_position=(0, lh * 32),
                            skip_group_check=True,
                        )

            def emit_av_pack(it):
                qc, kt = divmod(it, NKT)
                if kt == 0:
                    aggs[qc] = agg_tag_tile(f"agg{qc}")
                agg, at = aggs[qc], ats.pop(it)
                for half in range(QC // HQ):
                    hcols = slice(half * HQ, (half + 1) * HQ)
                    first = kt == 0 and (half * HQ) % HF == 0
                    for lh in range(HPC):
                        esl = slice(lh * HD, (lh + 1) * HD)
                        nc.tensor.matmul(
                            agg[esl, hcols],
                            V_sb[:, kt, esl],
                            at[:, lh, hcols],
                            start=first,
                            stop=(kt == NKT - 1),
                            tile_position=(0, lh * HD),
                            skip_group_check=True,
                        )

            def emit_epilogue(qc):
                qcols = slice(qc * QC, (qc + 1) * QC)
                zps, agg = zpss.pop(qc), aggs.pop(qc)
                zsb = smallp.tile([33, QC], F16, tag="zsb", name=f"zsb{qc}")
                nc.vector.tensor_copy(zsb, zps)
                for lh in range(HPC):
                    nc.sync.dma_start(
                        out=zout[lh, qcols], in_=zsb[lh * 32 : lh * 32 + 1, :]
                    )
                # split the agg eviction so the first half's DMA overlaps the
                # second half's PSUM->SBUF copy
                osb = outp.tile([P, QC], F16, tag="osb", name=f"osb_{qc}")
                for half in range(2):
                    hsl = slice(half * HF, (half + 1) * HF)
                    nc.vector.tensor_copy(osb[:, hsl], agg[:, hsl])
                    nc.sync.dma_start(
                        out=outT[:, qc * QC + half * HF : qc * QC + (half + 1) * HF],
                        in_=osb[:, hsl],
                    )

            s_cur = [s_tile(lh, f"s_0_{lh}") for lh in range(HPC)]
            for lh in range(HPC):
                emit_s(0, lh, s_cur[lh])

            z_next = 0
            av_next = 0
            epi_done = 0
            v_ps = [None, None]  # borrowed psum tiles for the V sub-chunks

            def try_epi():
                nonlocal epi_done
                while (
                    epi_done < NQC
                    and z_next > epi_done * NKT + NKT - 1
                    and av_next > epi_done * NKT + NKT - 1
                ):
                    emit_epilogue(epi_done)
                    epi_done += 1

            def drain_packs(it):
                """Emit pending Z/AV packs.  Z runs ZLAG behind (gated only
                by the K-B/Q-B psum borrow of the z banks); AV runs AVLAG
                behind (the V projection borrows the agg banks) and catches
                up from it>=14 so both cross the qc boundary together.  A
                pack may not enter a new qc until that qc's predecessor
                epilogue is out (its first matmul re-claims the banks the
                epilogue eviction reads)."""
                nonlocal z_next, av_next
                if it >= 30:
                    zlim, zq = it, 2
                elif it >= 26:
                    zlim, zq = it - 1, 2
                else:
                    zlim, zq = it - ZLAG, 1
                for _ in range(zq):
                    j = z_next
                    if j >= NIT or j > zlim:
                        break
                    if j % NKT == 0 and j > 0 and epi_done < j // NKT:
                        break
                    emit_z_pack(j)
                    z_next += 1
                    try_epi()
                if it >= 30:
                    alim, aq = it, 2
                elif it >= 24:
                    alim, aq = it - 1, 2
                elif it >= 14:
                    alim, aq = it - 2, (2 if it % 2 == 0 else 1)
                else:
                    alim, aq = it - AVLAG, 1
                for _ in range(aq):
                    j = av_next
                    if j >= NIT or j > alim:
                        break
                    if j % NKT == 0 and j > 0 and epi_done < j // NKT:
                        break
                    emit_av_pack(j)
                    av_next += 1
                    try_epi()

            for it in range(NIT):
                last = it == NIT - 1
                if it + MLEAD < NIT:
                    masks[it + MLEAD] = emit_mask_dma(it + MLEAD)
                mt = masks.pop(it)
                et = etp.tile([P, HPC, QC], F16, tag="et", name=f"et_{it}")
                at = atp.tile([P, HPC, QC], F16, tag="at", name=f"at_{it}")
                ets[it], ats[it] = et, at
                s_nxt = (
                    [s_tile(lh, f"s_{it + 1}_{lh}") for lh in range(HPC)]
                    if not last
                    else None
                )
                for lh in range(HPC):
                    nc.scalar.activation(
                        et[:, lh, :],
                        s_cur[lh],
                        mybir.ActivationFunctionType.Exp,
                        scale=SCALING,
                    )
                    nc.vector.tensor_mul(at[:, lh, :], et[:, lh, :], mt[:, lh, :])
                    # S^T for the next iteration reuses this head's PSUM
                    # banks; emit right after the exp that frees them.
                    if not last:
                        emit_s(it + 1, lh, s_nxt[lh])
                s_cur = s_nxt

                # deferred projections, wedged into the PE's slack BEFORE the
                # Z/AV packs (the packs may wait on late masks; the wedges
                # must not queue behind them).  Each wedge's inputs are in
                # SBUF just before the PE's FIFO reaches it, so it never
                # blocks the S matmuls emitted after it.
                if it == 1:
                    emit_proj(KT_sb, wk_sb, bqk_sb[:, 1:2], kbs, 1,
                              z_tag_tile("ps_kB", shape=(P, QC)))
                elif it == 3:
                    emit_proj(QT_sb, wq_sb, bqk_sb[:, 0:1], qbs, 1,
                              z_tag_tile("ps_qB", shape=(P, QC)))
                elif it == 4:
                    v_ps[0] = agg_tag_tile("ps_vA")
                    emit_v_chunk(0, v_ps[0])
                elif it == 5:
                    emit_v_chunk(1, v_ps[0])
                elif it == 6:
                    v_ps[1] = agg_tag_tile("ps_vB")
                    emit_v_chunk(2, v_ps[1])
                elif it == 7:
                    emit_v_chunk(3, v_ps[1])

                drain_packs(it)

            it = NIT
            while z_next < NIT or av_next < NIT:
                drain_packs(it)
                it += 1

    nc.compile()
    return nc


# ---------------------------------------------------------------------------
# Host side
# ---------------------------------------------------------------------------
def _prep_in_maps(q, k, v, mask_head, pearson_matrix, Wq, bq, Wk, bk, Wv, bv):
    f = np.float32
    q = np.asarray(q, f)
    k = np.asarray(k, f)
    v = np.asarray(v, f)
    mask_head = np.asarray(mask_head, f)
    Wq = np.asarray(Wq, f)
    Wk = np.asarray(Wk, f)
    Wv = np.asarray(Wv, f)
    bq = np.asarray(bq, f).reshape(D)
    bk = np.asarray(bk, f).reshape(D)
    bv = np.asarray(bv, f).reshape(D)

    # Only the diagonal of pearson is used by the computation.
    pm = np.asarray(pearson_matrix)
    diag = np.ascontiguousarray(np.diagonal(pm, axis1=-2, axis2=-1)).astype(f)

    def _thalves(x):
        """x [n, d] -> [2, d, n/2] fp16, each column-half contiguous."""
        dst = _alloc((2, D, QC), np.float16)
        xT = x.T
        np.copyto(dst[0], xT[:, 0:QC])
        np.copyto(dst[1], xT[:, QC:N])
        return dst

    qT = [_thalves(q[b]) for b in range(B)]
    kTt = [_thalves(k[b]) for b in range(B)]
    vTt = [_thalves(v[b]) for b in range(B)]
    onesd = np.ones((P, P), np.float16)

    def wtile(W, esl):
        # [D, E] -> [P, NCC, E] with d = c*P + p
        wT = np.ascontiguousarray(W[esl, :].T.astype(np.float16))
        return np.ascontiguousarray(wT.reshape(NCC, P, E).transpose(1, 0, 2))

    # Per-(b,h) mask, transposed to [k, q], diag-folded, tiled to the exact
    # per-iteration consumption order: [qc, kt, k, lh, q].
    maskt_all = _alloc((B, H // HPC, NQC, NKT, P, HPC, QC), np.float16)
    for b in range(B):
        for h in range(H):
            md = mask_head[b, h].T * diag[b, h][:, None]  # [k, q] f32
            tiled = md.reshape(NKT, P, NQC, QC).transpose(2, 0, 1, 3)
            maskt_all[b, h // HPC, :, :, :, h % HPC, :] = tiled

    in_maps = []
    for c in range(NCORES):
        b = c // (NCORES // B)
        h0 = HPC * (c % (NCORES // B))
        esl = slice(h0 * HD, (h0 + HPC) * HD)
        bqk = np.ascontiguousarray(
            np.stack([bq[esl], bk[esl]], axis=1).astype(f)
        )
        bvb = np.ascontiguousarray(
            np.broadcast_to(bv[esl][None, :], (P, E)).astype(f)
        )
        in_maps.append(
            {
                "qTh": qT[b],
                "kTh": kTt[b],
                "vTh": vTt[b],
                "wq": wtile(Wq, esl),
                "wk": wtile(Wk, esl),
                "wv": wtile(Wv, esl),
                "bqk": bqk,
                "bvb": bvb,
                "onesd": onesd,
                "maskt": maskt_all[b, h0 // HPC],
            }
        )
    return in_maps


_NC_CACHE = None
LAST_RESULT = None  # BassKernelResults of the most recent run (for profiling)


def kernel(**inputs) -> np.ndarray:
    global _NC_CACHE, LAST_RESULT
    _install_shims()
    from concourse.bass_utils import run_bass_kernel_spmd

    if _NC_CACHE is None:
        _NC_CACHE = build_nc()
    nc = _NC_CACHE

    in_maps = _prep_in_maps(**inputs)

    trace = bool(int(os.environ.get("KERNEL_TRACE", "0")))
    kwargs = {}
    if trace:
        kwargs["trace"] = True
        tmpdir = os.environ.get("KERNEL_TRACE_DIR")
        if tmpdir:
            kwargs["tmpdir"] = tmpdir
    res = run_bass_kernel_spmd(nc, in_maps, list(range(NCORES)), **kwargs)
    LAST_RESULT = res

    out = _alloc((B, N, D), np.float32)
    for c in range(NCORES):
        b = c // (NCORES // B)
        h0 = HPC * (c % (NCORES // B))
        aggT = np.asarray(res.results[c]["outT"], np.float32)  # (E, N)
        z = np.asarray(res.results[c]["zout"], np.float32)  # (HPC, N)
        out[b, :, h0 * HD : (h0 + HPC) * HD] = (
            aggT / np.repeat(z, HD, axis=0)
        ).T
    return out


# revision 45
# speedup vs baseline: 1.0327x; 1.0011x over previous
"""MultiHeadCrossAttention kernel for 8 Trainium2 NeuronCores.

Reference computation (b=2, nq=nk=2048, d_model=512, h=8, hd=64):
    Q = split_heads(q @ Wq.T + bq); K, V likewise
    S = Q K^T * hd^-0.5 ; A = softmax(S, -1) * mask_head * diag(pearson)
    out = merge_heads(A @ V)

Sharding: 16 (batch, head) pairs -> 2 heads of one batch per core.

Only the *diagonal* of pearson_matrix is used, so it is extracted on the
host and folded into the mask.  The mask is transposed, diag-folded,
tiled to the exact per-iteration consumption order and cast to fp16 on
the host, so every device-side mask DMA is one contiguous 512 KiB read.

All matmul operands are fp16 (PE runs 16-bit at 4x the fp32 rate, and
fp16's 10 mantissa bits keep the error ~1e-3, far under the 2e-2 gate).
PSUM accumulation stays fp32.

Device layout per core ("k on partitions, q on free axis"):
    S^T[k,q]   = sum_d K^T[d,k] Q^T[d,q]       (TensorE, d=64 contraction)
    E^T        = exp(SCALING * S^T)            (ScalarE, PSUM->SBUF fp16)
    Z[q]      += ones^T @ E^T                  (TensorE, PSUM-accumulated,
                                                heads packed in col groups
                                                0/32 -> run concurrently)
    A^T        = E^T * maskT_folded            (VectorE, fp16 2x mode)
    agg^T[e,q]+= sum_k V[k,e] A^T[k,q]         (TensorE, PSUM-accumulated,
                                                heads col-packed 0/64)

The scalar-engine exp stream (~1.0us per [128,1024] tile, 64 tiles) is
the critical path; everything else is scheduled around keeping it dense:

 * PSUM S tiles double-buffer per head so the S matmuls for k-tile kt+1
   are emitted right after the exp that frees the banks.
 * Z/AV matmul packs are emitted several iterations late (ZLAG/AVLAG) so
   the PE never blocks the S->exp chain on mask/projection stragglers.
 * The K(cols 1024:)/Q(cols 1024:)/V projections are interleaved into
   the early/mid iterations, borrowing the agg/z PSUM tags before the
   (lagged) first Z/AV accumulations claim them.
 * A burst of tiny warm-up matmuls keeps the PE HAM un-throttled while
   the k/q DMAs are in flight.
 * GpSimd is never given a data op, avoiding its ~6us one-time ucode
   IRAM load at kernel start.

The device returns unnormalized agg^T (128 rows = 2 heads x 64 dims) and
Z; the host divides, transposes and concatenates the 8 per-core slices.
"""

import ctypes
import os
import sys
import types

import numpy as np

import concourse.bacc as bacc
import concourse.bass as bass
import concourse.tile as tile
from concourse import mybir
from concourse.vector_clock import ScopedClock

F32 = mybir.dt.float32
F16 = mybir.dt.float16

B = 2
H = 8
N = 2048  # nq == nk
D = 512
HD = 64
HPC = 2  # heads per core
E = HPC * HD  # 128 output dims per core
SCALING = HD ** (-0.5)
NCORES = 8
P = 128
QC = 1024  # q super-chunk (2 per core)
NQC = N // QC
NKT = N // P  # 16 k tiles
NIT = NQC * NKT  # 32 global iterations
HF = 512  # matmul free-dim chunk (one PSUM bank)
NCC = D // P  # 4 contraction chunks for the projections

ZLAG = 5  # Z packs for iteration i are emitted at iteration i+ZLAG
AVLAG = 9  # AV pack lag (V projection borrows the agg banks early on)
MLEAD = 2  # mask DMA for iteration i is emitted at iteration i-MLEAD
NWARM = 56  # PE warm-up matmuls
HQ = 256  # Z/AV pack matmul free-dim (small => less PE head-of-line blocking)


# ---------------------------------------------------------------------------
# Page faults are extremely slow in this sandbox (~ms each); MAP_POPULATE
# prefaults an allocation in one syscall, ~100x faster for big arrays.
# ---------------------------------------------------------------------------
_libc = ctypes.CDLL(None, use_errno=True)
_libc.mmap.restype = ctypes.c_void_p
_libc.mmap.argtypes = [
    ctypes.c_void_p,
    ctypes.c_size_t,
    ctypes.c_int,
    ctypes.c_int,
    ctypes.c_int,
    ctypes.c_long,
]


def _alloc(shape, dtype=np.float32):
    nbytes = int(np.prod(shape)) * np.dtype(dtype).itemsize
    nbytes = (nbytes + 4095) & ~4095
    p = _libc.mmap(None, nbytes, 0x3, 0x02 | 0x20 | 0x8000, -1, 0)  # RW, PRIV|ANON|POPULATE
    if p in (None, ctypes.c_void_p(-1).value):
        return np.empty(shape, dtype)
    buf = (ctypes.c_byte * nbytes).from_address(p)
    return np.frombuffer(buf, dtype=dtype, count=int(np.prod(shape))).reshape(shape)


def _tcopy16(src):
    """Contiguous fp16 transposed copy of a 2-D array into prefaulted memory."""
    dst = _alloc((src.shape[1], src.shape[0]), np.float16)
    np.copyto(dst, src.T)
    return dst


# ---------------------------------------------------------------------------
# Environment shim: walrus in this container rejects >1 sync wait on
# CTRL-class instructions (NoOp/Drain), but TileContext's kernel-tail drain
# carries one wait per live semaphore.  Re-emit them as individual wait_ge
# instructions (one wait each) before a bare drain.
# ---------------------------------------------------------------------------
def _drain_and_barrier(self, tick_clock, wait_clock):
    probe = mybir.InstNoOp(
        name="wait_probe", ins=[], outs=[], engine=mybir.EngineType.SP
    )
    wait_clock.add_sem_waits(probe, ScopedClock({None: tick_clock.global_clock}))
    waits = list(probe.sync_info.on_wait) if probe.sync_info else []
    allocated = self.sems.allocated()
    by_name = {}
    for k, h in allocated.items():
        by_name[getattr(h, "name", str(k))] = h
    for w in waits:
        h = by_name.get(w.ant_name)
        assert h is not None, (w.ant_name, sorted(by_name))
        self.nc.sync.wait_ge(h, w.wait_value)
    self.nc.sync.drain()
    self.nc.all_engine_barrier()
    popped = self.nc._tile_sem_poison_stack.pop()
    assert popped is self._sem_poison
    self.nc.clear_and_free_semaphores(list(allocated.values()))
    self.nc.all_engine_barrier()


def _install_shims():
    tile.TileContext._drain_and_barrier = _drain_and_barrier
    if "antenv.axon_hooks" not in sys.modules:
        try:
            from trn_agent_boot.trn_boot import _ntff_profile_via_ctypes

            mod = types.ModuleType("antenv.axon_hooks")
            hook = _ntff_profile_via_ctypes("/opt/axon/libaxon_pjrt.so")
            mod.get_axon_ntff_profile_hook = lambda: hook
            mod.set_axon_ntff_profile_hook = lambda h: None
            sys.modules["antenv.axon_hooks"] = mod
        except Exception:
            pass


# ---------------------------------------------------------------------------
# Device kernel (one Bass program, SPMD over 8 cores; shards via in_maps)
# ---------------------------------------------------------------------------
def build_nc() -> bass.Bass:
    nc = bacc.Bacc("TRN2", target_bir_lowering=False, debug=False)

    # [half, d, n-half]: each column-half contiguous for full-rate DMA
    qTh = nc.dram_tensor("qTh", [2, D, QC], F16, kind="ExternalInput")
    kTh = nc.dram_tensor("kTh", [2, D, QC], F16, kind="ExternalInput")
    vTh = nc.dram_tensor("vTh", [2, D, QC], F16, kind="ExternalInput")
    # weights pre-tiled on the host to the SBUF layout [p, c, e]
    wq = nc.dram_tensor("wq", [P, NCC, E], F16, kind="ExternalInput")
    wk = nc.dram_tensor("wk", [P, NCC, E], F16, kind="ExternalInput")
    wv = nc.dram_tensor("wv", [P, NCC, E], F16, kind="ExternalInput")
    bqk = nc.dram_tensor("bqk", [E, 2], F32, kind="ExternalInput")  # bq|bk
    bvb = nc.dram_tensor("bvb", [P, E], F32, kind="ExternalInput")  # bv bcast
    onesd = nc.dram_tensor("onesd", [P, P], F16, kind="ExternalInput")
    # maskt[qc, kt, k, lh, q] = mask[b, h0+lh, qc*QC+q, kt*P+k]
    #                          * diag(pearson)[b, h0+lh, kt*P+k]
    maskt = nc.dram_tensor("maskt", [NQC, NKT, P, HPC, QC], F16, kind="ExternalInput")
    outT = nc.dram_tensor("outT", [E, N], F16, kind="ExternalOutput")
    # softmax denominators, normalization happens on the host
    zout = nc.dram_tensor("zout", [HPC, N], F16, kind="ExternalOutput")

    with tile.TileContext(nc) as tc:
        with (
            tc.tile_pool(name="consts", bufs=1) as consts,
            tc.tile_pool(name="persist", bufs=1) as persist,
            tc.tile_pool(name="ps", bufs=1, space="PSUM") as ps,
            tc.tile_pool(name="qkv", bufs=1) as qkv,
            tc.tile_pool(name="et", bufs=ZLAG + 3) as etp,
            tc.tile_pool(name="at", bufs=AVLAG + 2) as atp,
            tc.tile_pool(name="mask", bufs=MLEAD + 4) as maskp,
            tc.tile_pool(name="small", bufs=2) as smallp,
            tc.tile_pool(name="outp", bufs=2) as outp,
        ):
            def s_tile(sl, name):
                return ps.tile([P, QC], F32, tag=f"s{sl}", name=name)

            def agg_tag_tile(name, shape=(P, QC)):
                return ps.tile(list(shape), F32, tag="agg", name=name)

            def z_tag_tile(name, shape=(33, QC)):
                return ps.tile(list(shape), F32, tag="z", name=name)

            # ---- constants & input DMAs (ordering = HWDGE FIFO order) -----
            ones = consts.tile([P, P], F16, tag="ones")
            nc.sync.dma_start(out=ones, in_=onesd[:, :])
            wk_sb = consts.tile([P, NCC, E], F16, tag="wk")
            nc.sync.dma_start(out=wk_sb, in_=wk[:, :, :])
            bqk_sb = consts.tile([E, 2], F32, tag="bqk")
            nc.sync.dma_start(out=bqk_sb, in_=bqk[:, :])

            # k/q staged as column-half chunks so the projections feeding the
            # exp stream's first iterations wait on 1 MiB, not 2 MiB, each.
            # One 1-MiB DMA per half (not 4x 256 KiB) for full DMA rate.
            ka_t = qkv.tile([P, NCC, QC], F16, tag="ka")
            qa_t = qkv.tile([P, NCC, QC], F16, tag="qa")
            kb_t = qkv.tile([P, NCC, QC], F16, tag="kb")
            qb_t = qkv.tile([P, NCC, QC], F16, tag="qb")
            v_t = qkv.tile([P, NCC, 2, QC], F16, tag="v")
            nc.sync.dma_start(out=ka_t, in_=kTh[0].rearrange("(c p) n -> p c n", p=P))
            nc.sync.dma_start(out=qa_t, in_=qTh[0].rearrange("(c p) n -> p c n", p=P))
            wq_sb = consts.tile([P, NCC, E], F16, tag="wq")
            nc.sync.dma_start(out=wq_sb, in_=wq[:, :, :])
            nc.sync.dma_start(out=kb_t, in_=kTh[1].rearrange("(c p) n -> p c n", p=P))
            nc.sync.dma_start(out=qb_t, in_=qTh[1].rearrange("(c p) n -> p c n", p=P))
            wv_sb = consts.tile([P, NCC, E], F16, tag="wv")
            bvb_sb = consts.tile([P, E], F32, tag="bvb")
            for vh in range(2):
                nc.sync.dma_start(
                    out=v_t[:, :, vh, :],
                    in_=vTh[vh].rearrange("(c p) n -> p c n", p=P),
                )
            nc.sync.dma_start(out=wv_sb, in_=wv[:, :, :])
            nc.sync.dma_start(out=bvb_sb, in_=bvb[:, :])
            kas = [ka_t[:, c, :] for c in range(NCC)]
            qas = [qa_t[:, c, :] for c in range(NCC)]
            kbs = [kb_t[:, c, :] for c in range(NCC)]
            qbs = [qb_t[:, c, :] for c in range(NCC)]
            vhs = [v_t[:, c, vh, :] for c in range(NCC) for vh in range(2)]

            # Preload the exp spline table during the DMAs.
            scratch = consts.tile([P, 8], F32, tag="scratch")
            nc.scalar.activation(scratch, ones[:, 0:8], mybir.ActivationFunctionType.Exp)

            # PE warm-up: a dense burst of tiny matmuls while k/q stream in,
            # so the HAM clock gate opens before the projections run.
            warm_ps = s_tile(0, "warm_ps")
            for i in range(NWARM):
                nc.tensor.matmul(
                    warm_ps[0:1, 0:P],
                    ones[:, 0:1],
                    ones[:, :],
                    start=True,
                    stop=True,
                    skip_group_check=True,
                )

            # ---- K/Q projections, first halves (cols 0:1024) --------------
            # [e, n] = sum_c w[c, e] * xT[c, n]; bias added during the
            # PSUM->SBUF eviction on VectorE (per-partition scalar operand).
            QT_sb = persist.tile([E, N], F16, tag="QT")  # [e, n] 2 heads x 64
            KT_sb = persist.tile([E, N], F16, tag="KT")
            V_sb = persist.tile([P, NKT, E], F16, tag="V")  # [k%128, kt, e]

            def emit_proj(dst, w_sb, bias_ap, srcs, jcol, pst):
                for half in range(QC // HF):
                    cols = slice(half * HF, (half + 1) * HF)
                    for c in range(NCC):
                        nc.tensor.matmul(
                            pst[:, half * HF : (half + 1) * HF],
                            w_sb[:, c, :],
                            srcs[c][:, cols],
                            start=(c == 0),
                            stop=(c == NCC - 1),
                        )
                for half in range(QC // HF):
                    cols = slice(jcol * QC + half * HF, jcol * QC + (half + 1) * HF)
                    nc.vector.tensor_scalar_add(
                        dst[:, cols], pst[:, half * HF : (half + 1) * HF], bias_ap
                    )

            ps_kA = s_tile(0, "ps_kA")
            for half in range(QC // HF):
                cols = slice(half * HF, (half + 1) * HF)
                for c in range(NCC):
                    nc.tensor.matmul(
                        ps_kA[:, cols],
                        wk_sb[:, c, :],
                        kas[c][:, cols],
                        start=(c == 0),
                        stop=(c == NCC - 1),
                    )
            nc.vector.tensor_scalar_add(
                KT_sb[:, 0:HF], ps_kA[:, 0:HF], bqk_sb[:, 1:2]
            )
            emit_proj(QT_sb, wq_sb, bqk_sb[:, 0:1], qas, 0, s_tile(1, "ps_qA"))
            nc.vector.tensor_scalar_add(
                KT_sb[:, HF:QC], ps_kA[:, HF:QC], bqk_sb[:, 1:2]
            )

            def emit_v_chunk(sub, pst):
                """V natural layout: [n, e] = sum_c vT[c, n] * w[c, e].
                One sub-chunk = 4 k-tiles into half of a [128,1024] psum."""
                for t4 in range(4):
                    t = sub * 4 + t4
                    col = (sub % 2) * 4 + t4
                    for c in range(NCC):
                        nc.tensor.matmul(
                            pst[:, col * E : (col + 1) * E],
                            vhs[2 * c + t // 8][:, (t % 8) * P : (t % 8 + 1) * P],
                            wv_sb[:, c, :],
                            start=(c == 0),
                            stop=(c == NCC - 1),
                        )
                for t4 in range(4):
                    t = sub * 4 + t4
                    col = (sub % 2) * 4 + t4
                    nc.vector.tensor_add(
                        V_sb[:, t, :], pst[:, col * E : (col + 1) * E], bvb_sb
                    )

            # ---- attention: 32 software-pipelined iterations ---------------
            def emit_s(it, lh, s_ps):
                qc, kt = divmod(it, NKT)
                kcols = slice(kt * P, (kt + 1) * P)
                hsl = slice(lh * HD, (lh + 1) * HD)
                for half in range(QC // HF):
                    rcols = slice(qc * QC + half * HF, qc * QC + (half + 1) * HF)
                    nc.tensor.matmul(
                        s_ps[:, half * HF : (half + 1) * HF],
                        KT_sb[hsl, kcols],
                        QT_sb[hsl, rcols],
                        start=True,
                        stop=True,
                        tile_position=(lh * HD, 0),
                    )

            ets = {}
            ats = {}
            aggs = {}
            zpss = {}

            def emit_mask_dma(it):
                qc, kt = divmod(it, NKT)
                mt = maskp.tile([P, HPC, QC], F16, tag="mt", name=f"mt_{it}")
                nc.sync.dma_start(out=mt, in_=maskt[qc, kt])
                return mt

            masks = {it: emit_mask_dma(it) for it in range(MLEAD)}

            def emit_z_pack(it):
                qc, kt = divmod(it, NKT)
                if kt == 0:
                    zpss[qc] = z_tag_tile(f"z{qc}")
                zps, et = zpss[qc], ets.pop(it)
                for half in range(QC // HQ):
                    hcols = slice(half * HQ, (half + 1) * HQ)
                    # start=True clears the has_written bits of the WHOLE
                    # psum bank (for the written partitions), so only the
                    # first sub-bank group may carry it; the second group's
                    # kt==0 matmul overwrites thanks to the cleared bits.
                    first = kt == 0 and (half * HQ) % HF == 0
                    for lh in range(HPC):
                        nc.tensor.matmul(
                            zps[lh * 32 : lh * 32 + 1, hcols],
                            ones[:, 0:1],
                            et[:, lh, hcols],
                            start=first,
                            stop=(kt == NKT - 1),
                            tile_position=(0, lh * 32),
                            skip_group_check=True,
                        )

            def emit_av_pack(it):
                qc, kt = divmod(it, NKT)
                if kt == 0:
                    aggs[qc] = agg_tag_tile(f"agg{qc}")
                agg, at = aggs[qc], ats.pop(it)
                for half in range(QC // HQ):
                    hcols = slice(half * HQ, (half + 1) * HQ)
                    first = kt == 0 and (half * HQ) % HF == 0
                    for lh in range(HPC):
                        esl = slice(lh * HD, (lh + 1) * HD)
                        nc.tensor.matmul(
                            agg[esl, hcols],
                            V_sb[:, kt, esl],
                            at[:, lh, hcols],
                            start=first,
                            stop=(kt == NKT - 1),
                            tile_position=(0, lh * HD),
                            skip_group_check=True,
                        )

            def emit_epilogue(qc):
                qcols = slice(qc * QC, (qc + 1) * QC)
                zps, agg = zpss.pop(qc), aggs.pop(qc)
                zsb = smallp.tile([33, QC], F16, tag="zsb", name=f"zsb{qc}")
                nc.vector.tensor_copy(zsb, zps)
                for lh in range(HPC):
                    nc.sync.dma_start(
                        out=zout[lh, qcols], in_=zsb[lh * 32 : lh * 32 + 1, :]
                    )
                # split the agg eviction so the first half's DMA overlaps the
                # second half's PSUM->SBUF copy
                osb = outp.tile([P, QC], F16, tag="osb", name=f"osb_{qc}")
                for half in range(2):
                    hsl = slice(half * HF, (half + 1) * HF)
                    nc.vector.tensor_copy(osb[:, hsl], agg[:, hsl])
                    nc.sync.dma_start(
                        out=outT[:, qc * QC + half * HF : qc * QC + (half + 1) * HF],
                        in_=osb[:, hsl],
                    )

            s_cur = [s_tile(lh, f"s_0_{lh}") for lh in range(HPC)]
            for lh in range(HPC):
                emit_s(0, lh, s_cur[lh])

            z_next = 0
            av_next = 0
            epi_done = 0
            v_ps = [None, None]  # borrowed psum tiles for the V sub-chunks

            def try_epi():
                nonlocal epi_done
                while (
                    epi_done < NQC
                    and z_next > epi_done * NKT + NKT - 1
                    and av_next > epi_done * NKT + NKT - 1
                ):
                    emit_epilogue(epi_done)
                    epi_done += 1

            def drain_packs(it):
                """Emit pending Z/AV packs.  Z runs ZLAG behind (gated only
                by the K-B/Q-B psum borrow of the z banks); AV runs AVLAG
                behind (the V projection borrows the agg banks) and catches
                up from it>=14 so both cross the qc boundary together.  A
                pack may not enter a new qc until that qc's predecessor
                epilogue is out (its first matmul re-claims the banks the
                epilogue eviction reads)."""
                nonlocal z_next, av_next
                if it >= 30:
                    zlim, zq = it, 2
                elif it >= 26:
                    zlim, zq = it - 1, 2
                else:
                    zlim, zq = it - ZLAG, 1
                for _ in range(zq):
                    j = z_next
                    if j >= NIT or j > zlim:
                        break
                    if j % NKT == 0 and j > 0 and epi_done < j // NKT:
                        break
                    emit_z_pack(j)
                    z_next += 1
                    try_epi()
                if it >= 30:
                    alim, aq = it, 2
                elif it >= 24:
                    alim, aq = it - 1, 2
                elif it >= 14:
                    alim, aq = it - 2, (2 if it % 2 == 0 else 1)
                else:
                    alim, aq = it - AVLAG, 1
                for _ in range(aq):
                    j = av_next
                    if j >= NIT or j > alim:
                        break
                    if j % NKT == 0 and j > 0 and epi_done < j // NKT:
                        break
                    emit_av_pack(j)
                    av_next += 1
                    try_epi()

            for it in range(NIT):
                last = it == NIT - 1
                if it + MLEAD < NIT:
                    masks[it + MLEAD] = emit_mask_dma(it + MLEAD)
                mt = masks.pop(it)
                et = etp.tile([P, HPC, QC], F16, tag="et", name=f"et_{it}")
                at = atp.tile([P, HPC, QC], F16, tag="at", name=f"at_{it}")
                ets[it], ats[it] = et, at
                s_nxt = (
                    [s_tile(lh, f"s_{it + 1}_{lh}") for lh in range(HPC)]
                    if not last
                    else None
                )
                for lh in range(HPC):
                    nc.scalar.activation(
                        et[:, lh, :],
                        s_cur[lh],
                        mybir.ActivationFunctionType.Exp,
                        scale=SCALING,
                    )
                    nc.vector.tensor_mul(at[:, lh, :], et[:, lh, :], mt[:, lh, :])
                    # S^T for the next iteration reuses this head's PSUM
                    # banks; emit right after the exp that frees them.
                    if not last:
                        emit_s(it + 1, lh, s_nxt[lh])
                s_cur = s_nxt

                # deferred projections, wedged into the PE's slack BEFORE the
                # Z/AV packs (the packs may wait on late masks; the wedges
                # must not queue behind them).  Each wedge's inputs are in
                # SBUF just before the PE's FIFO reaches it, so it never
                # blocks the S matmuls emitted after it.
                if it == 1:
                    emit_proj(KT_sb, wk_sb, bqk_sb[:, 1:2], kbs, 1,
                              z_tag_tile("ps_kB", shape=(P, QC)))
                elif it == 3:
                    emit_proj(QT_sb, wq_sb, bqk_sb[:, 0:1], qbs, 1,
                              z_tag_tile("ps_qB", shape=(P, QC)))
                elif it == 4:
                    v_ps[0] = agg_tag_tile("ps_vA")
                    emit_v_chunk(0, v_ps[0])
                elif it == 5:
                    emit_v_chunk(1, v_ps[0])
                elif it == 6:
                    v_ps[1] = agg_tag_tile("ps_vB")
                    emit_v_chunk(2, v_ps[1])
                elif it == 7:
                    emit_v_chunk(3, v_ps[1])

                drain_packs(it)

            it = NIT
            while z_next < NIT or av_next < NIT:
                drain_packs(it)
                it += 1

    nc.compile()
    return nc


# ---------------------------------------------------------------------------
# Host side
# ---------------------------------------------------------------------------
def _prep_in_maps(q, k, v, mask_head, pearson_matrix, Wq, bq, Wk, bk, Wv, bv):
    f = np.float32
    q = np.asarray(q, f)
    k = np.asarray(k, f)
    v = np.asarray(v, f)
    mask_head = np.asarray(mask_head, f)
    Wq = np.asarray(Wq, f)
    Wk = np.asarray(Wk, f)
    Wv = np.asarray(Wv, f)
    bq = np.asarray(bq, f).reshape(D)
    bk = np.asarray(bk, f).reshape(D)
    bv = np.asarray(bv, f).reshape(D)

    # Only the diagonal of pearson is used by the computation.
    pm = np.asarray(pearson_matrix)
    diag = np.ascontiguousarray(np.diagonal(pm, axis1=-2, axis2=-1)).astype(f)

    def _thalves(x):
        """x [n, d] -> [2, d, n/2] fp16, each column-half contiguous."""
        dst = _alloc((2, D, QC), np.float16)
        xT = x.T
        np.copyto(dst[0], xT[:, 0:QC])
        np.copyto(dst[1], xT[:, QC:N])
        return dst

    qT = [_thalves(q[b]) for b in range(B)]
    kTt = [_thalves(k[b]) for b in range(B)]
    vTt = [_thalves(v[b]) for b in range(B)]
    onesd = np.ones((P, P), np.float16)

    def wtile(W, esl):
        # [D, E] -> [P, NCC, E] with d = c*P + p
        wT = np.ascontiguousarray(W[esl, :].T.astype(np.float16))
        return np.ascontiguousarray(wT.reshape(NCC, P, E).transpose(1, 0, 2))

    # Per-(b,h) mask, transposed to [k, q], diag-folded, tiled to the exact
    # per-iteration consumption order: [qc, kt, k, lh, q].
    maskt_all = _alloc((B, H // HPC, NQC, NKT, P, HPC, QC), np.float16)
    for b in range(B):
        for h in range(H):
            md = mask_head[b, h].T * diag[b, h][:, None]  # [k, q] f32
            tiled = md.reshape(NKT, P, NQC, QC).transpose(2, 0, 1, 3)
            maskt_all[b, h // HPC, :, :, :, h % HPC, :] = tiled

    in_maps = []
    for c in range(NCORES):
        b = c // (NCORES // B)
        h0 = HPC * (c % (NCORES // B))
        esl = slice(h0 * HD, (h0 + HPC) * HD)
        bqk = np.ascontiguousarray(
            np.stack([bq[esl], bk[esl]], axis=1).astype(f)
        )
        bvb = np.ascontiguousarray(
            np.broadcast_to(bv[esl][None, :], (P, E)).astype(f)
        )
        in_maps.append(
            {
                "qTh": qT[b],
                "kTh": kTt[b],
                "vTh": vTt[b],
                "wq": wtile(Wq, esl),
                "wk": wtile(Wk, esl),
                "wv": wtile(Wv, esl),
                "bqk": bqk,
                "bvb": bvb,
                "onesd": onesd,
                "maskt": maskt_all[b, h0 // HPC],
            }
        )
    return in_maps


_NC_CACHE = None
LAST_RESULT = None  # BassKernelResults of the most recent run (for profiling)


def kernel(**inputs) -> np.ndarray:
    global _NC_CACHE, LAST_RESULT
    _install_shims()
    from concourse.bass_utils import run_bass_kernel_spmd

    if _NC_CACHE is None:
        _NC_CACHE = build_nc()
    nc = _NC_CACHE

    in_maps = _prep_in_maps(**inputs)

    trace = bool(int(os.environ.get("KERNEL_TRACE", "0")))
    kwargs = {}
    if trace:
        kwargs["trace"] = True
        tmpdir = os.environ.get("KERNEL_TRACE_DIR")
        if tmpdir:
            kwargs["tmpdir"] = tmpdir
    res = run_bass_kernel_spmd(nc, in_maps, list(range(NCORES)), **kwargs)
    LAST_RESULT = res

    out = _alloc((B, N, D), np.float32)
    for c in range(NCORES):
        b = c // (NCORES // B)
        h0 = HPC * (c % (NCORES // B))
        aggT = np.asarray(res.results[c]["outT"], np.float32)  # (E, N)
        z = np.asarray(res.results[c]["zout"], np.float32)  # (HPC, N)
        out[b, :, h0 * HD : (h0 + HPC) * HD] = (
            aggT / np.repeat(z, HD, axis=0)
        ).T
    return out
